# revision 10
# baseline (speedup 1.0000x reference)
"""Trainium2 Bass kernel for nn_MessageAggregationAttention.

Shards B=256 graphs across 8 NeuronCores (32 graphs each). Each core:
  - indirect-DMA gathers its query rows and incoming-message rows from a
    replicated (edge_attr + zero-row) table,
  - runs per-graph 4-head attention (padded LQ=128 / LK=384) with the
    softmax computed in "transposed logits" layout (keys on partitions) so
    no attention-matrix transposes are needed,
  - applies out-proj + residual + FFN,
  - indirect-DMA scatters the result rows back to the full output.

Host side only builds small int32 index tables / masks and pre-transposed
weights; all heavy data movement and compute happens on device.
"""

import math

import numpy as np

import concourse.bass as bass
import concourse.mybir as mybir
from concourse import bacc
from concourse.bass_utils import run_bass_kernel_spmd
from concourse.masks import make_identity
from concourse.tile import TileContext

B, E, M, H, NH = 256, 16384, 65536, 128, 4
HD = H // NH               # 32
LQ, LK = 128, 384
NCORES = 8
G = B // NCORES            # 32 graphs per core
NQT = G * LQ // 128        # 32 query tiles of 128 slots
NKT = G * LK // 128        # 96 key tiles of 128 slots
XZ_ROWS = E + 128          # edge_attr + zero rows (row E.. are zeros)
MASK_VAL = -100.0          # exp(logit + MASK_VAL) == 0.0 in f32 for our logit range

f32 = mybir.dt.float32
i32 = mybir.dt.int32

AFT = mybir.ActivationFunctionType

# run_bass_kernel_spmd results from the last invocation (for test harness).
LAST_RESULTS = None
TRACE = False
TRACE_KW = {}


def _build_program():
    nc = bacc.Bacc("TRN2")

    xz = nc.dram_tensor("xz", [XZ_ROWS, H], f32, kind="ExternalInput")
    idxq_d = nc.dram_tensor("idxq", [128, NQT], i32, kind="ExternalInput")
    idxk_d = nc.dram_tensor("idxk", [128, NKT], i32, kind="ExternalInput")
    maskk_d = nc.dram_tensor("maskk", [128, NKT], f32, kind="ExternalInput")
    wqTz_d = nc.dram_tensor("wqTz", [H, 4 * H], f32, kind="ExternalInput")
    wkT_d = nc.dram_tensor("wkT", [H, H], f32, kind="ExternalInput")
    wvT_d = nc.dram_tensor("wvT", [H, H], f32, kind="ExternalInput")
    woT_d = nc.dram_tensor("woT", [H, H], f32, kind="ExternalInput")
    w1T_d = nc.dram_tensor("w1T", [H, 2 * H], f32, kind="ExternalInput")
    w2T_d = nc.dram_tensor("w2T", [2 * H, H], f32, kind="ExternalInput")
    bq_d = nc.dram_tensor("bqz", [H, 4], f32, kind="ExternalInput")
    bk_d = nc.dram_tensor("bkc", [H, 1], f32, kind="ExternalInput")
    bo_d = nc.dram_tensor("boc", [H, 1], f32, kind="ExternalInput")
    b1_d = nc.dram_tensor("b1c", [H, 2], f32, kind="ExternalInput")
    b2_d = nc.dram_tensor("b2c", [H, 1], f32, kind="ExternalInput")

    out = nc.dram_tensor("out", [XZ_ROWS, H], f32, kind="ExternalOutput")

    with TileContext(nc) as tc:
        with (
            tc.tile_pool(name="const", bufs=1) as constp,
            tc.tile_pool(name="xtok", bufs=4) as xtokp,
            tc.tile_pool(name="xkT", bufs=4) as xkTp,
            tc.tile_pool(name="qblk", bufs=1) as qblkp,
            tc.tile_pool(name="kv", bufs=3) as kvp,
            tc.tile_pool(name="exp", bufs=6) as expp,
            tc.tile_pool(name="attn", bufs=3) as attnp,
            tc.tile_pool(name="ffn", bufs=3) as ffnp,
            tc.tile_pool(name="ps_sm", bufs=3, space="PSUM") as ps_smp,
            tc.tile_pool(name="ps_big", bufs=2, space="PSUM") as ps_bigp,
            tc.tile_pool(name="ps_acc", bufs=1, space="PSUM") as ps_accp,
            tc.tile_pool(name="ps_den", bufs=1, space="PSUM") as ps_denp,
        ):
            # ---- constants ----
            ident = constp.tile([128, 128], f32)
            make_identity(nc, ident[:])
            ones_col = constp.tile([128, 1], f32)
            nc.vector.memset(ones_col[:], 1.0)
            ones_row = constp.tile([1, 32], f32)
            nc.vector.memset(ones_row[:], 1.0)

            def _load(shape, dram):
                t = constp.tile(shape, f32, tag=dram.name, name=dram.name + "_sb")
                nc.sync.dma_start(out=t[:], in_=dram[:])
                return t

            wqTz = _load([H, 4 * H], wqTz_d)
            wkT = _load([H, H], wkT_d)
            wvT = _load([H, H], wvT_d)
            woT = _load([H, H], woT_d)
            w1T = _load([H, 2 * H], w1T_d)
            w2T_a = constp.tile([128, H], f32, tag="w2Ta")
            w2T_b = constp.tile([128, H], f32, tag="w2Tb")
            nc.sync.dma_start(out=w2T_a[:], in_=w2T_d[0:128, :])
            nc.sync.dma_start(out=w2T_b[:], in_=w2T_d[128:256, :])
            bqz = _load([H, 4], bq_d)
            bkc = _load([H, 1], bk_d)
            boc = _load([H, 1], bo_d)
            b1c = _load([H, 2], b1_d)
            b2c = _load([H, 1], b2_d)
            maskk = _load([128, NKT], maskk_d)
            idxq = constp.tile([128, NQT], i32, tag="idxq")
            nc.sync.dma_start(out=idxq[:], in_=idxq_d[:])
            idxk = constp.tile([128, NKT], i32, tag="idxk")
            nc.sync.dma_start(out=idxk[:], in_=idxk_d[:])

            # ---- persistent per-phase activations (eight 512-col blocks) ----
            xqT_blk = [constp.tile([128, 512], f32, tag=f"xqT{b}", name=f"xqT{b}") for b in range(8)]
            qTz = constp.tile([128, G, 512], f32, tag="qTz", name="qTz")
            ar_blk = [constp.tile([128, 512], f32, tag=f"ar{b}", name=f"arb{b}") for b in range(8)]
            fin_blk = [constp.tile([128, 512], f32, tag=f"fin{b}", name=f"fin{b}") for b in range(8)]

            # ---- Q path: gather -> transpose -> xqT blocks ----
            for qt in range(NQT):
                xq_tok = xtokp.tile([128, 128], f32, tag="xq_tok")
                nc.gpsimd.indirect_dma_start(
                    out=xq_tok[:],
                    out_offset=None,
                    in_=xz[:],
                    in_offset=bass.IndirectOffsetOnAxis(
                        ap=idxq[:, qt : qt + 1], axis=0
                    ),
                )
                ps = ps_smp.tile([128, 128], f32, tag="ps_tr")
                nc.tensor.transpose(out=ps[:], in_=xq_tok[:], identity=ident[:])
                nc.vector.tensor_copy(
                    out=xqT_blk[qt // 4][:, (qt % 4) * 128 : (qt % 4 + 1) * 128],
                    in_=ps[:],
                )

            # ---- Q projection: per head (masked weights -> zero-blocked qTz) ----
            for h in range(4):
                for blk in range(8):
                    ps = ps_bigp.tile([128, 512], f32, tag="ps_big")
                    nc.tensor.matmul(
                        out=ps[:], lhsT=wqTz[:, h * 128 : (h + 1) * 128],
                        rhs=xqT_blk[blk][:], start=True, stop=True,
                    )
                    nc.vector.tensor_scalar_add(
                        out=qTz[:, 4 * blk : 4 * blk + 4, h * 128 : (h + 1) * 128],
                        in0=ps[:].rearrange("p (g q) -> p g q", q=128),
                        scalar1=bqz[:, h : h + 1],
                    )

            # ---- per graph: K/V path + attention ----
            for g in range(G):
                kT = kvp.tile([128, LK], f32, tag="kT")
                v_t = [kvp.tile([128, 128], f32, tag=f"v{t}", name=f"vt{t}") for t in range(3)]
                for t in range(3):
                    kt = g * 3 + t
                    xk_tok = xtokp.tile([128, 128], f32, tag="xk_tok")
                    nc.gpsimd.indirect_dma_start(
                        out=xk_tok[:],
                        out_offset=None,
                        in_=xz[:],
                        in_offset=bass.IndirectOffsetOnAxis(
                            ap=idxk[:, kt : kt + 1], axis=0
                        ),
                    )
                    ps = ps_smp.tile([128, 128], f32, tag="ps_tr")
                    nc.tensor.transpose(out=ps[:], in_=xk_tok[:], identity=ident[:])
                    xkT = xkTp.tile([128, 128], f32, tag="xkT")
                    nc.vector.tensor_copy(out=xkT[:], in_=ps[:])
                    # K projection (feature-major)
                    psk = ps_smp.tile([128, 128], f32, tag="ps_tr")
                    nc.tensor.matmul(
                        out=psk[:], lhsT=wkT[:], rhs=xkT[:], start=True, stop=True
                    )
                    nc.scalar.activation(
                        out=kT[:, t * 128 : (t + 1) * 128], in_=psk[:],
                        func=AFT.Identity, bias=bkc[:, 0:1],
                    )
                    # V projection (token-major: lhsT = xkT)
                    psv = ps_smp.tile([128, 128], f32, tag="ps_tr")
                    nc.tensor.matmul(
                        out=psv[:], lhsT=xkT[:], rhs=wvT[:], start=True, stop=True
                    )
                    nc.vector.tensor_copy(out=v_t[t][:], in_=psv[:])

                # attention for graph g
                ctx_ps = ps_accp.tile([128, 128], f32, tag="ctx")
                den_ps = ps_denp.tile([1, 512], f32, tag="den")
                for t in range(3):
                    lg_ps = ps_bigp.tile([128, 512], f32, tag="ps_big")
                    nc.tensor.matmul(
                        out=lg_ps[:],
                        lhsT=kT[:, t * 128 : (t + 1) * 128],
                        rhs=qTz[:, g, :],
                        start=True, stop=True,
                    )
                    ex = expp.tile([128, 512], f32, tag="exp")
                    kt = g * 3 + t
                    nc.scalar.activation(
                        out=ex[:], in_=lg_ps[:], func=AFT.Exp,
                        bias=maskk[:, kt : kt + 1],
                    )
                    nc.tensor.matmul(
                        out=den_ps[:], lhsT=ones_col[:], rhs=ex[:],
                        start=(t == 0), stop=(t == 2), skip_group_check=True,
                    )
                    for h in range(4):
                        nc.tensor.matmul(
                            out=ctx_ps[32 * h : 32 * (h + 1), :],
                            lhsT=v_t[t][:, 32 * h : 32 * (h + 1)],
                            rhs=ex[:, h * 128 : (h + 1) * 128],
                            start=(t == 0), stop=(t == 2), skip_group_check=True,
                            tile_position=(0, 32 * h),
                        )
                rden = attnp.tile([1, 512], f32, tag="rden")
                nc.vector.reciprocal(out=rden[:], in_=den_ps[:])
                bc_ps = ps_accp.tile([128, 128], f32, tag="bcast")
                for h in range(4):
                    nc.tensor.matmul(
                        out=bc_ps[32 * h : 32 * (h + 1), :],
                        lhsT=ones_row[:],
                        rhs=rden[:, h * 128 : (h + 1) * 128],
                        start=True, stop=True,
                        tile_position=(0, 32 * h),
                    )
                bc_sb = attnp.tile([128, 128], f32, tag="bc_sb")
                nc.scalar.copy(out=bc_sb[:], in_=bc_ps[:])
                ctxn = attnp.tile([128, 128], f32, tag="ctxn")
                nc.vector.tensor_mul(out=ctxn[:], in0=ctx_ps[:], in1=bc_sb[:])
                # out-proj + bias, then residual with xqT
                po = ps_smp.tile([128, 128], f32, tag="ps_tr")
                nc.tensor.matmul(
                    out=po[:], lhsT=woT[:], rhs=ctxn[:], start=True, stop=True
                )
                ao = attnp.tile([128, 128], f32, tag="ao")
                nc.scalar.activation(
                    out=ao[:], in_=po[:], func=AFT.Identity, bias=boc[:, 0:1]
                )
                qc = (g % 4) * 128
                nc.vector.tensor_add(
                    out=ar_blk[g // 4][:, qc : qc + 128],
                    in0=ao[:],
                    in1=xqT_blk[g // 4][:, qc : qc + 128],
                )

            # ---- FFN (batched over 512-col blocks) ----
            for blk in range(8):
                pa = ps_bigp.tile([128, 512], f32, tag="ps_big")
                nc.tensor.matmul(
                    out=pa[:], lhsT=w1T[:, 0:128], rhs=ar_blk[blk][:],
                    start=True, stop=True,
                )
                ra = ffnp.tile([128, 512], f32, tag="ra")
                nc.scalar.activation(
                    out=ra[:], in_=pa[:], func=AFT.Relu, bias=b1c[:, 0:1]
                )
                pb = ps_bigp.tile([128, 512], f32, tag="ps_big")
                nc.tensor.matmul(
                    out=pb[:], lhsT=w1T[:, 128:256], rhs=ar_blk[blk][:],
                    start=True, stop=True,
                )
                rb = ffnp.tile([128, 512], f32, tag="rb")
                nc.scalar.activation(
                    out=rb[:], in_=pb[:], func=AFT.Relu, bias=b1c[:, 1:2]
                )
                p2 = ps_bigp.tile([128, 512], f32, tag="ps_big")
                nc.tensor.matmul(
                    out=p2[:], lhsT=w2T_a[:], rhs=ra[:], start=True, stop=False,
                    skip_group_check=True,
                )
                nc.tensor.matmul(
                    out=p2[:], lhsT=w2T_b[:], rhs=rb[:], start=False, stop=True,
                    skip_group_check=True,
                )
                f2 = ffnp.tile([128, 512], f32, tag="f2")
                nc.scalar.activation(
                    out=f2[:], in_=p2[:], func=AFT.Identity, bias=b2c[:, 0:1]
                )
                nc.vector.tensor_add(
                    out=fin_blk[blk][:], in0=f2[:], in1=ar_blk[blk][:]
                )

            # ---- transpose back + scatter ----
            for qt in range(NQT):
                ps = ps_smp.tile([128, 128], f32, tag="ps_tr")
                nc.tensor.transpose(
                    out=ps[:],
                    in_=fin_blk[qt // 4][:, (qt % 4) * 128 : (qt % 4 + 1) * 128],
                    identity=ident[:],
                )
                ftok = xtokp.tile([128, 128], f32, tag="ftok")
                nc.vector.tensor_copy(out=ftok[:], in_=ps[:])
                nc.gpsimd.indirect_dma_start(
                    out=out[:],
                    out_offset=bass.IndirectOffsetOnAxis(
                        ap=idxq[:, qt : qt + 1], axis=0
                    ),
                    in_=ftok[:],
                    in_offset=None,
                )
    nc.finalize()
    return nc


_NC_CACHE = None


def kernel(edge_index, edge_attr, incoming_edges_list, incoming_edges_batch,
           edge_batch, in_proj_w, in_proj_b, out_proj_w, out_proj_b,
           w1, b1, w2, b2):
    global _NC_CACHE, LAST_RESULTS

    edge_attr = np.asarray(edge_attr, np.float32)
    edge_batch = np.asarray(edge_batch, np.int64)
    incoming_edges_list = np.asarray(incoming_edges_list, np.int64)
    incoming_edges_batch = np.asarray(incoming_edges_batch, np.int64)

    # ---- host prep: index tables, masks, weights ----
    xz = np.zeros((XZ_ROWS, H), np.float32)
    xz[:E] = edge_attr

    cnt_q = np.bincount(edge_batch, minlength=B)
    st_q = np.zeros(B + 1, np.int64)
    np.cumsum(cnt_q, out=st_q[1:])
    cnt_k = np.bincount(incoming_edges_batch, minlength=B)
    st_k = np.zeros(B + 1, np.int64)
    np.cumsum(cnt_k, out=st_k[1:])
    assert cnt_q.max() <= LQ and cnt_k.max() <= LK and cnt_k.min() >= 1

    # [B, LQ] global edge row per (graph, slot); E for padding
    pos_q = np.arange(LQ)[None, :]
    idxq_full = np.where(
        pos_q < cnt_q[:, None], st_q[:B, None] + pos_q, E
    ).astype(np.int32)
    pos_k = np.arange(LK)[None, :]
    gath = np.full((B, LK), E, np.int64)
    valid = pos_k < cnt_k[:, None]
    flat_idx = (st_k[:B, None] + np.minimum(pos_k, cnt_k[:, None] - 1))
    gath[valid] = incoming_edges_list[flat_idx[valid]]
    idxk_full = gath.astype(np.int32)
    maskk_full = np.where(valid, 0.0, MASK_VAL).astype(np.float32)

    s = 1.0 / math.sqrt(HD)
    wq, wk, wv = in_proj_w[:H], in_proj_w[H:2 * H], in_proj_w[2 * H:]
    bq, bk, bv = in_proj_b[:H], in_proj_b[H:2 * H], in_proj_b[2 * H:]
    wqT = np.ascontiguousarray((wq * s).T, np.float32)
    wqTz = np.zeros((H, 4 * H), np.float32)
    bqz = np.zeros((H, 4), np.float32)
    for h in range(4):
        wqTz[:, h * H + 32 * h : h * H + 32 * (h + 1)] = \
            wqT[:, 32 * h : 32 * (h + 1)]
        bqz[32 * h : 32 * (h + 1), h] = (bq * s)[32 * h : 32 * (h + 1)]
    wkT = np.ascontiguousarray(wk.T, np.float32)
    wvT = np.ascontiguousarray(wv.T, np.float32)
    woT = np.ascontiguousarray(out_proj_w.T, np.float32)
    w1T = np.ascontiguousarray(w1.T, np.float32)          # [H, 2H]
    w2T = np.ascontiguousarray(w2.T, np.float32)          # [2H, H]
    bkc = np.ascontiguousarray(bk[:, None], np.float32)
    boc = np.ascontiguousarray(
        (out_proj_b + out_proj_w @ bv)[:, None], np.float32
    )
    b1c = np.ascontiguousarray(b1.reshape(2, H).T, np.float32)
    b2c = np.ascontiguousarray(b2[:, None], np.float32)

    shared = dict(xz=xz, wqTz=wqTz, wkT=wkT, wvT=wvT, woT=woT, w1T=w1T,
                  w2T=w2T, bqz=bqz, bkc=bkc, boc=boc, b1c=b1c, b2c=b2c)
    in_maps = []
    for c in range(NCORES):
        gs = slice(c * G, (c + 1) * G)
        # [G, L] -> [128, n_tiles]: tile j of 128 slots -> column j
        idxq_c = np.ascontiguousarray(
            idxq_full[gs].reshape(NQT, 128).T)
        idxk_c = np.ascontiguousarray(
            idxk_full[gs].reshape(NKT, 128).T)
        maskk_c = np.ascontiguousarray(
            maskk_full[gs].reshape(NKT, 128).T)
        in_maps.append(dict(shared, idxq=idxq_c, idxk=idxk_c, maskk=maskk_c))

    if _NC_CACHE is None:
        _NC_CACHE = _build_program()
    res = run_bass_kernel_spmd(
        _NC_CACHE, in_maps, core_ids=list(range(NCORES)),
        trace=TRACE, **TRACE_KW,
    )
    LAST_RESULTS = res

    out_full = np.zeros((E, H), np.float32)
    for c in range(NCORES):
        s0, s1 = int(st_q[c * G]), int(st_q[(c + 1) * G])
        out_full[s0:s1] = res.results[c]["out"][s0:s1]
    return out_full


# revision 11
# speedup vs baseline: 1.4046x; 1.4046x over previous
"""Trainium2 Bass kernel for nn_MessageAggregationAttention.

Shards B=256 graphs across 8 NeuronCores (32 graphs each). Each core:
  - indirect-DMA gathers its query rows and incoming-message rows from a
    replicated (edge_attr + zero-row) table,
  - runs per-graph 4-head attention (padded LQ=128 / LK=384) with the
    softmax computed in "transposed logits" layout (keys on partitions) so
    no attention-matrix transposes are needed,
  - applies out-proj + residual + FFN,
  - indirect-DMA scatters the result rows back to the full output.

Host side only builds small int32 index tables / masks and pre-transposed
weights; all heavy data movement and compute happens on device.
"""

import math

import ml_dtypes
import numpy as np

import concourse.bass as bass
import concourse.mybir as mybir
from concourse import bacc
from concourse.bass_utils import run_bass_kernel_spmd
from concourse.masks import make_identity
from concourse.tile import TileContext

B, E, M, H, NH = 256, 16384, 65536, 128, 4
HD = H // NH               # 32
LQ, LK = 128, 384
NCORES = 8
G = B // NCORES            # 32 graphs per core
NQT = G * LQ // 128        # 32 query tiles of 128 slots
NKT = G * LK // 128        # 96 key tiles of 128 slots
XZ_ROWS = E + 128          # edge_attr + zero rows (row E.. are zeros)
MASK_VAL = -100.0          # exp(logit + MASK_VAL) == 0.0 in f32 for our logit range

f32 = mybir.dt.float32
bf16 = mybir.dt.bfloat16
i32 = mybir.dt.int32

AFT = mybir.ActivationFunctionType

# run_bass_kernel_spmd results from the last invocation (for test harness).
LAST_RESULTS = None
TRACE = False
TRACE_KW = {}


def _build_program():
    nc = bacc.Bacc("TRN2")

    xz = nc.dram_tensor("xz", [XZ_ROWS, H], f32, kind="ExternalInput")
    idxq_d = nc.dram_tensor("idxq", [128, NQT], i32, kind="ExternalInput")
    idxk_d = nc.dram_tensor("idxk", [128, NKT], i32, kind="ExternalInput")
    maskk_d = nc.dram_tensor("maskk", [128, NKT], f32, kind="ExternalInput")
    wqTz_d = nc.dram_tensor("wqTz", [H, 4 * H], f32, kind="ExternalInput")
    wkT_d = nc.dram_tensor("wkT", [H, H], bf16, kind="ExternalInput")
    wvT_d = nc.dram_tensor("wvT", [H, H], bf16, kind="ExternalInput")
    woT_d = nc.dram_tensor("woT", [H, H], bf16, kind="ExternalInput")
    w1T_d = nc.dram_tensor("w1T", [H, 2 * H], bf16, kind="ExternalInput")
    w2T_d = nc.dram_tensor("w2T", [2 * H, H], bf16, kind="ExternalInput")
    bq_d = nc.dram_tensor("bqz", [H, 4], f32, kind="ExternalInput")
    bk_d = nc.dram_tensor("bkc", [H, 1], f32, kind="ExternalInput")
    bo_d = nc.dram_tensor("boc", [H, 1], f32, kind="ExternalInput")
    b1_d = nc.dram_tensor("b1c", [H, 2], f32, kind="ExternalInput")
    b2_d = nc.dram_tensor("b2c", [H, 1], f32, kind="ExternalInput")

    out = nc.dram_tensor("out", [XZ_ROWS, H], f32, kind="ExternalOutput")

    with TileContext(nc) as tc:
        with (
            tc.tile_pool(name="const", bufs=1) as constp,
            tc.tile_pool(name="xtok", bufs=4) as xtokp,
            tc.tile_pool(name="xkT", bufs=4) as xkTp,
            tc.tile_pool(name="qblk", bufs=1) as qblkp,
            tc.tile_pool(name="kv", bufs=3) as kvp,
            tc.tile_pool(name="exp", bufs=6) as expp,
            tc.tile_pool(name="attn", bufs=3) as attnp,
            tc.tile_pool(name="ffn", bufs=3) as ffnp,
            tc.tile_pool(name="ps_sm", bufs=2, space="PSUM") as ps_smp,
            tc.tile_pool(name="ps_big", bufs=3, space="PSUM") as ps_bigp,
            tc.tile_pool(name="ps_acc", bufs=1, space="PSUM") as ps_accp,
            tc.tile_pool(name="ps_den", bufs=1, space="PSUM") as ps_denp,
        ):
            # ---- constants ----
            ident = constp.tile([128, 128], f32)
            make_identity(nc, ident[:])
            ones_col = constp.tile([128, 1], bf16)
            nc.vector.memset(ones_col[:], 1.0)
            ones_row = constp.tile([1, 32], f32)
            nc.vector.memset(ones_row[:], 1.0)

            def _load(shape, dram, dt=f32):
                t = constp.tile(shape, dt, tag=dram.name, name=dram.name + "_sb")
                nc.sync.dma_start(out=t[:], in_=dram[:])
                return t

            wqTz = _load([H, 4 * H], wqTz_d)
            wkT = _load([H, H], wkT_d, bf16)
            wvT = _load([H, H], wvT_d, bf16)
            woT = _load([H, H], woT_d, bf16)
            w1T = _load([H, 2 * H], w1T_d, bf16)
            w2T_a = constp.tile([128, H], bf16, tag="w2Ta")
            w2T_b = constp.tile([128, H], bf16, tag="w2Tb")
            nc.sync.dma_start(out=w2T_a[:], in_=w2T_d[0:128, :])
            nc.sync.dma_start(out=w2T_b[:], in_=w2T_d[128:256, :])
            bqz = _load([H, 4], bq_d)
            bkc = _load([H, 1], bk_d)
            boc = _load([H, 1], bo_d)
            b1c = _load([H, 2], b1_d)
            b2c = _load([H, 1], b2_d)
            maskk = _load([128, NKT], maskk_d)
            idxq = constp.tile([128, NQT], i32, tag="idxq")
            nc.sync.dma_start(out=idxq[:], in_=idxq_d[:])
            idxk = constp.tile([128, NKT], i32, tag="idxk")
            nc.sync.dma_start(out=idxk[:], in_=idxk_d[:])

            # ---- persistent per-phase activations (eight 512-col blocks) ----
            xqT_blk = [constp.tile([128, 512], f32, tag=f"xqT{b}", name=f"xqT{b}") for b in range(8)]
            qTz = constp.tile([128, G, 512], bf16, tag="qTz", name="qTz")
            ar_blk = [constp.tile([128, 512], f32, tag=f"ar{b}", name=f"arb{b}") for b in range(8)]
            fin_blk = [constp.tile([128, 512], f32, tag=f"fin{b}", name=f"fin{b}") for b in range(8)]

            # ---- Q path: gather -> transpose -> xqT blocks ----
            for qt in range(NQT):
                xq_tok = xtokp.tile([128, 128], f32, tag="xq_tok")
                nc.gpsimd.indirect_dma_start(
                    out=xq_tok[:],
                    out_offset=None,
                    in_=xz[:],
                    in_offset=bass.IndirectOffsetOnAxis(
                        ap=idxq[:, qt : qt + 1], axis=0
                    ),
                )
                ps = ps_smp.tile([128, 128], f32, tag="ps_tr")
                nc.tensor.transpose(out=ps[:], in_=xq_tok[:], identity=ident[:])
                nc.vector.tensor_copy(
                    out=xqT_blk[qt // 4][:, (qt % 4) * 128 : (qt % 4 + 1) * 128],
                    in_=ps[:],
                )

            # ---- Q projection: per head (masked weights -> zero-blocked qTz) ----
            for h in range(4):
                for blk in range(8):
                    ps = ps_bigp.tile([128, 512], f32, tag="ps_big")
                    nc.tensor.matmul(
                        out=ps[:], lhsT=wqTz[:, h * 128 : (h + 1) * 128],
                        rhs=xqT_blk[blk][:], start=True, stop=True,
                    )
                    nc.vector.tensor_scalar_add(
                        out=qTz[:, 4 * blk : 4 * blk + 4, h * 128 : (h + 1) * 128],
                        in0=ps[:].rearrange("p (g q) -> p g q", q=128),
                        scalar1=bqz[:, h : h + 1],
                    )

            # ---- per graph: K/V path + attention ----
            for g in range(G):
                kT = kvp.tile([128, LK], bf16, tag="kT")
                v_t = [kvp.tile([128, 128], bf16, tag=f"v{t}", name=f"vt{t}") for t in range(3)]
                for t in range(3):
                    kt = g * 3 + t
                    xk_tok = xtokp.tile([128, 128], f32, tag="xk_tok")
                    nc.gpsimd.indirect_dma_start(
                        out=xk_tok[:],
                        out_offset=None,
                        in_=xz[:],
                        in_offset=bass.IndirectOffsetOnAxis(
                            ap=idxk[:, kt : kt + 1], axis=0
                        ),
                    )
                    ps = ps_smp.tile([128, 128], f32, tag="ps_tr")
                    nc.tensor.transpose(out=ps[:], in_=xk_tok[:], identity=ident[:])
                    xkT = xkTp.tile([128, 128], bf16, tag="xkT")
                    nc.vector.tensor_copy(out=xkT[:], in_=ps[:])
                    # K projection (feature-major)
                    psk = ps_smp.tile([128, 128], f32, tag="ps_tr")
                    nc.tensor.matmul(
                        out=psk[:], lhsT=wkT[:], rhs=xkT[:], start=True, stop=True
                    )
                    nc.scalar.activation(
                        out=kT[:, t * 128 : (t + 1) * 128], in_=psk[:],
                        func=AFT.Identity, bias=bkc[:, 0:1],
                    )
                    # V projection (token-major: lhsT = xkT)
                    psv = ps_smp.tile([128, 128], f32, tag="ps_tr")
                    nc.tensor.matmul(
                        out=psv[:], lhsT=xkT[:], rhs=wvT[:], start=True, stop=True
                    )
                    nc.vector.tensor_copy(out=v_t[t][:], in_=psv[:])

                # attention for graph g
                ctx_ps = ps_accp.tile([128, 128], f32, tag="ctx")
                den_ps = ps_denp.tile([1, 512], f32, tag="den")
                for t in range(3):
                    lg_ps = ps_bigp.tile([128, 512], f32, tag="ps_big")
                    nc.tensor.matmul(
                        out=lg_ps[:],
                        lhsT=kT[:, t * 128 : (t + 1) * 128],
                        rhs=qTz[:, g, :],
                        start=True, stop=True,
                    )
                    ex = expp.tile([128, 512], bf16, tag="exp")
                    kt = g * 3 + t
                    nc.scalar.activation(
                        out=ex[:], in_=lg_ps[:], func=AFT.Exp,
                        bias=maskk[:, kt : kt + 1],
                    )
                    nc.tensor.matmul(
                        out=den_ps[:], lhsT=ones_col[:], rhs=ex[:],
                        start=(t == 0), stop=(t == 2), skip_group_check=True,
                    )
                    for h in range(4):
                        nc.tensor.matmul(
                            out=ctx_ps[32 * h : 32 * (h + 1), :],
                            lhsT=v_t[t][:, 32 * h : 32 * (h + 1)],
                            rhs=ex[:, h * 128 : (h + 1) * 128],
                            start=(t == 0), stop=(t == 2), skip_group_check=True,
                            tile_position=(0, 32 * h),
                        )
                rden = attnp.tile([1, 512], f32, tag="rden")
                nc.vector.reciprocal_approx_fast(out=rden[:], in_=den_ps[:])
                bc_ps = ps_accp.tile([128, 128], f32, tag="bcast")
                for h in range(4):
                    nc.tensor.matmul(
                        out=bc_ps[32 * h : 32 * (h + 1), :],
                        lhsT=ones_row[:],
                        rhs=rden[:, h * 128 : (h + 1) * 128],
                        start=True, stop=True,
                        tile_position=(0, 32 * h),
                    )
                bc_sb = attnp.tile([128, 128], f32, tag="bc_sb")
                nc.scalar.copy(out=bc_sb[:], in_=bc_ps[:])
                ctxn = attnp.tile([128, 128], bf16, tag="ctxn")
                nc.vector.tensor_mul(out=ctxn[:], in0=ctx_ps[:], in1=bc_sb[:])
                # out-proj + bias, then residual with xqT
                po = ps_smp.tile([128, 128], f32, tag="ps_tr")
                nc.tensor.matmul(
                    out=po[:], lhsT=woT[:], rhs=ctxn[:], start=True, stop=True
                )
                ao = attnp.tile([128, 128], f32, tag="ao")
                nc.scalar.activation(
                    out=ao[:], in_=po[:], func=AFT.Identity, bias=boc[:, 0:1]
                )
                qc = (g % 4) * 128
                nc.vector.tensor_add(
                    out=ar_blk[g // 4][:, qc : qc + 128],
                    in0=ao[:],
                    in1=xqT_blk[g // 4][:, qc : qc + 128],
                )

            # ---- FFN (batched over 512-col blocks) ----
            arbf_blk = [constp.tile([128, 512], bf16, tag=f"arbf{b}", name=f"arbf{b}")
                        for b in range(8)]
            for blk in range(8):
                nc.vector.tensor_copy(out=arbf_blk[blk][:], in_=ar_blk[blk][:])
                pa = ps_bigp.tile([128, 512], f32, tag="ps_big")
                nc.tensor.matmul(
                    out=pa[:], lhsT=w1T[:, 0:128], rhs=arbf_blk[blk][:],
                    start=True, stop=True,
                )
                ra = ffnp.tile([128, 512], bf16, tag="ra")
                nc.scalar.activation(
                    out=ra[:], in_=pa[:], func=AFT.Relu, bias=b1c[:, 0:1]
                )
                pb = ps_bigp.tile([128, 512], f32, tag="ps_big")
                nc.tensor.matmul(
                    out=pb[:], lhsT=w1T[:, 128:256], rhs=arbf_blk[blk][:],
                    start=True, stop=True,
                )
                rb = ffnp.tile([128, 512], bf16, tag="rb")
                nc.scalar.activation(
                    out=rb[:], in_=pb[:], func=AFT.Relu, bias=b1c[:, 1:2]
                )
                p2 = ps_bigp.tile([128, 512], f32, tag="ps_big")
                nc.tensor.matmul(
                    out=p2[:], lhsT=w2T_a[:], rhs=ra[:], start=True, stop=False,
                    skip_group_check=True,
                )
                nc.tensor.matmul(
                    out=p2[:], lhsT=w2T_b[:], rhs=rb[:], start=False, stop=True,
                    skip_group_check=True,
                )
                f2 = ffnp.tile([128, 512], f32, tag="f2")
                nc.scalar.activation(
                    out=f2[:], in_=p2[:], func=AFT.Identity, bias=b2c[:, 0:1]
                )
                nc.vector.tensor_add(
                    out=fin_blk[blk][:], in0=f2[:], in1=ar_blk[blk][:]
                )

            # ---- transpose back + scatter ----
            for qt in range(NQT):
                ps = ps_smp.tile([128, 128], f32, tag="ps_tr")
                nc.tensor.transpose(
                    out=ps[:],
                    in_=fin_blk[qt // 4][:, (qt % 4) * 128 : (qt % 4 + 1) * 128],
                    identity=ident[:],
                )
                ftok = xtokp.tile([128, 128], f32, tag="ftok")
                nc.vector.tensor_copy(out=ftok[:], in_=ps[:])
                nc.gpsimd.indirect_dma_start(
                    out=out[:],
                    out_offset=bass.IndirectOffsetOnAxis(
                        ap=idxq[:, qt : qt + 1], axis=0
                    ),
                    in_=ftok[:],
                    in_offset=None,
                )
    nc.finalize()
    return nc


_NC_CACHE = None


def kernel(edge_index, edge_attr, incoming_edges_list, incoming_edges_batch,
           edge_batch, in_proj_w, in_proj_b, out_proj_w, out_proj_b,
           w1, b1, w2, b2):
    global _NC_CACHE, LAST_RESULTS

    edge_attr = np.asarray(edge_attr, np.float32)
    edge_batch = np.asarray(edge_batch, np.int64)
    incoming_edges_list = np.asarray(incoming_edges_list, np.int64)
    incoming_edges_batch = np.asarray(incoming_edges_batch, np.int64)

    # ---- host prep: index tables, masks, weights ----
    xz = np.zeros((XZ_ROWS, H), np.float32)
    xz[:E] = edge_attr

    cnt_q = np.bincount(edge_batch, minlength=B)
    st_q = np.zeros(B + 1, np.int64)
    np.cumsum(cnt_q, out=st_q[1:])
    cnt_k = np.bincount(incoming_edges_batch, minlength=B)
    st_k = np.zeros(B + 1, np.int64)
    np.cumsum(cnt_k, out=st_k[1:])
    assert cnt_q.max() <= LQ and cnt_k.max() <= LK and cnt_k.min() >= 1

    # [B, LQ] global edge row per (graph, slot); E for padding
    pos_q = np.arange(LQ)[None, :]
    idxq_full = np.where(
        pos_q < cnt_q[:, None], st_q[:B, None] + pos_q, E
    ).astype(np.int32)
    pos_k = np.arange(LK)[None, :]
    gath = np.full((B, LK), E, np.int64)
    valid = pos_k < cnt_k[:, None]
    flat_idx = (st_k[:B, None] + np.minimum(pos_k, cnt_k[:, None] - 1))
    gath[valid] = incoming_edges_list[flat_idx[valid]]
    idxk_full = gath.astype(np.int32)
    maskk_full = np.where(valid, 0.0, MASK_VAL).astype(np.float32)

    s = 1.0 / math.sqrt(HD)
    wq, wk, wv = in_proj_w[:H], in_proj_w[H:2 * H], in_proj_w[2 * H:]
    bq, bk, bv = in_proj_b[:H], in_proj_b[H:2 * H], in_proj_b[2 * H:]
    wqT = np.ascontiguousarray((wq * s).T, np.float32)
    wqTz = np.zeros((H, 4 * H), np.float32)
    bqz = np.zeros((H, 4), np.float32)
    for h in range(4):
        wqTz[:, h * H + 32 * h : h * H + 32 * (h + 1)] = \
            wqT[:, 32 * h : 32 * (h + 1)]
        bqz[32 * h : 32 * (h + 1), h] = (bq * s)[32 * h : 32 * (h + 1)]
    bft = ml_dtypes.bfloat16
    wkT = np.ascontiguousarray(wk.T.astype(bft))
    wvT = np.ascontiguousarray(wv.T.astype(bft))
    woT = np.ascontiguousarray(out_proj_w.T.astype(bft))
    w1T = np.ascontiguousarray(w1.T.astype(bft))          # [H, 2H]
    w2T = np.ascontiguousarray(w2.T.astype(bft))          # [2H, H]
    bkc = np.ascontiguousarray(bk[:, None], np.float32)
    boc = np.ascontiguousarray(
        (out_proj_b + out_proj_w @ bv)[:, None], np.float32
    )
    b1c = np.ascontiguousarray(b1.reshape(2, H).T, np.float32)
    b2c = np.ascontiguousarray(b2[:, None], np.float32)

    shared = dict(xz=xz, wqTz=wqTz, wkT=wkT, wvT=wvT, woT=woT, w1T=w1T,
                  w2T=w2T, bqz=bqz, bkc=bkc, boc=boc, b1c=b1c, b2c=b2c)
    in_maps = []
    for c in range(NCORES):
        gs = slice(c * G, (c + 1) * G)
        # [G, L] -> [128, n_tiles]: tile j of 128 slots -> column j
        idxq_c = np.ascontiguousarray(
            idxq_full[gs].reshape(NQT, 128).T)
        idxk_c = np.ascontiguousarray(
            idxk_full[gs].reshape(NKT, 128).T)
        maskk_c = np.ascontiguousarray(
            maskk_full[gs].reshape(NKT, 128).T)
        in_maps.append(dict(shared, idxq=idxq_c, idxk=idxk_c, maskk=maskk_c))

    if _NC_CACHE is None:
        _NC_CACHE = _build_program()
    res = run_bass_kernel_spmd(
        _NC_CACHE, in_maps, core_ids=list(range(NCORES)),
        trace=TRACE, **TRACE_KW,
    )
    LAST_RESULTS = res

    out_full = np.zeros((E, H), np.float32)
    for c in range(NCORES):
        s0, s1 = int(st_q[c * G]), int(st_q[(c + 1) * G])
        out_full[s0:s1] = res.results[c]["out"][s0:s1]
    return out_full
